# revision 5
# baseline (speedup 1.0000x reference)
"""Hetero-GAT (2-layer) + MLP head on 8 Trainium2 NeuronCores.

Strategy: destination-sharded edge processing. For each relation, edges are
grouped per dst node (CSR), dst nodes sorted by degree (desc) into tiles of
128. Sub-tile j holds the j-th edge of each of the 128 dst nodes in the tile
(one indirect-DMA gather of 128 rows from the source node-type's extended
feature table [x | as-cols | ad-cols]). Attention weights e = exp(lrelu(as+ad))
are computed per tile in batched vector ops; the weighted sum over each dst's
edges is a strided multiply + reduce. The per-tile result is normalized,
transposed on the TensorEngine, multiplied by W_src (bias folded in via a
rank-1 matmul), and scattered to a per-relation accumulator in natural local
order. A per-type merge pass sums relation accumulators, applies tanh, and
emits the next layer's extended tables, which are all-gathered across the 8
cores. Layer 2 only computes the three node types the head consumes.
"""

import numpy as np

import concourse.bass as bass
import concourse.bacc as bacc
import concourse.mybir as mybir
import concourse.tile as tile
from concourse.bass_utils import run_bass_kernel_spmd
from concourse.masks import make_identity

P = 128
N_CORES = 8
HID = 128
OUT = 2
FEAT = 64
NEG_SLOPE = 0.2
NODE_SIZES = {
    "user": 100_000,
    "card": 150_000,
    "user_history": 100_000,
    "user_history_transaction": 400_000,
    "merchant": 20_000,
    "pending_transaction": 100_000,
}
EDGE_TYPES = [
    ("user", "owns", "card", 150_000),
    ("card", "belongs_to", "user", 150_000),
    ("user", "has", "user_history", 100_000),
    ("user_history", "belongs_to", "user", 100_000),
    ("user_history_transaction", "part_of", "user_history", 400_000),
    ("user_history_transaction", "paid_with", "card", 400_000),
    ("user_history_transaction", "made_at", "merchant", 400_000),
    ("card", "paid_for", "user_history_transaction", 400_000),
    ("merchant", "made", "user_history_transaction", 400_000),
    ("user_history", "reflects_on", "pending_transaction", 100_000),
    ("merchant", "selling", "pending_transaction", 100_000),
    ("user", "purchasing", "pending_transaction", 100_000),
]
HEAD_TYPES = ["pending_transaction", "user", "user_history"]
TYPES = list(NODE_SIZES)
SENT_AS = -1.0e9


def _k(s, r, d):
    return s + "__" + r + "__" + d


# relations used per layer (layer 2 only needs dst in HEAD_TYPES)
RELS_L = [
    [(s, r, d) for s, r, d, _ in EDGE_TYPES],
    [(s, r, d) for s, r, d, _ in EDGE_TYPES if d in HEAD_TYPES],
]


def _prep_graph(edges):
    """Per (relation, core): degree-sorted compact tiling + index columns.

    Returns meta dict:
      meta[rel] = dict(ntiles, maxdeg[t], idx[core] -> int32 [ncols, 128],
                       local ids (for scatter), etc.)  (layer-agnostic parts)
    """
    g = {}
    for s, r, d, _ in EDGE_TYPES:
        kk = _k(s, r, d)
        e = np.asarray(edges[kk])
        src = e[0].astype(np.int64)
        dst = e[1].astype(np.int64)
        nd = NODE_SIZES[d]
        sh = nd // N_CORES
        cores = []
        for c in range(N_CORES):
            m = (dst >= c * sh) & (dst < (c + 1) * sh)
            ds = dst[m] - c * sh
            ss = src[m]
            deg = np.bincount(ds, minlength=sh)
            nonempty = np.nonzero(deg)[0]
            order = nonempty[np.argsort(-deg[nonempty], kind="stable")]
            # CSR over local dst
            perm = np.argsort(ds, kind="stable")
            ss_sorted = ss[perm]
            starts = np.zeros(sh + 1, np.int64)
            np.cumsum(deg, out=starts[1:])
            cores.append(dict(order=order, deg=deg, starts=starts, ss=ss_sorted))
        ntiles = max((len(ci["order"]) + P - 1) // P for ci in cores)
        maxdeg = np.zeros(ntiles, np.int64)
        for ci in cores:
            o, dg = ci["order"], ci["deg"]
            for t in range((len(o) + P - 1) // P):
                maxdeg[t] = max(maxdeg[t], dg[o[t * P]])
        maxdeg = np.maximum(maxdeg, 1)
        g[(s, r, d)] = dict(cores=cores, ntiles=ntiles, maxdeg=maxdeg, sh=sh)
    return g


def _idx_columns(g, li):
    """Build per-core index column arrays for layer li.

    Column layout per (rel, tile): [ad_col, src cols x maxdeg[t], scat_col].
    Returns idxs[core][rel] -> int32 [128, ncols] and a per-rel col count.
    """
    def src_row(t, ids):
        # map global node id array -> ext table row (layer-dependent)
        n = NODE_SIZES[t]
        if li == 0:
            return ids
        sh = n // N_CORES
        return (ids // sh) * (sh + 2) + (ids % sh)

    def sent_row(t):
        n = NODE_SIZES[t]
        return n if li == 0 else (n // N_CORES)

    out = [dict() for _ in range(N_CORES)]
    for rel in RELS_L[li]:
        s, r, d = rel
        G = g[rel]
        ntiles, maxdeg, sh = G["ntiles"], G["maxdeg"], G["sh"]
        ncols = int(np.sum(maxdeg + 2))
        for c in range(N_CORES):
            ci = G["cores"][c]
            o, dg, starts, ss = ci["order"], ci["deg"], ci["starts"], ci["ss"]
            cols = np.full((ncols, P), -1, np.int64)
            col = 0
            for t in range(ntiles):
                md = int(maxdeg[t])
                dloc = o[t * P:(t + 1) * P]
                npad = len(dloc)
                # ad col: dst rows into ext table of type d
                adcol = np.full(P, sent_row(d), np.int64)
                if npad:
                    adcol[:npad] = src_row(d, dloc + c * sh)
                cols[col] = adcol
                col += 1
                # src cols
                sc = np.full((md, P), sent_row(s), np.int64)
                if npad:
                    st_ = starts[dloc]
                    dgp = dg[dloc]
                    for j in range(md):
                        mm = dgp > j
                        if mm.any():
                            sc[j, :npad][mm] = src_row(s, ss[st_[mm] + j])
                cols[col:col + md] = sc
                col += md
                # scatter col: local acc row (trash = sh)
                stc = np.full(P, sh, np.int64)
                if npad:
                    stc[:npad] = dloc
                cols[col] = stc
                col += 1
            assert col == ncols
            out[c][rel] = np.ascontiguousarray(cols.T).astype(np.int32)
    return out


def _ext1_tables(xs, params):
    """Layer-1 extended tables [N+1, FEAT + c_t] per type, host-built."""
    lp = params["layers"][0]
    tabs = {}
    offs = {}
    for t in TYPES:
        x = np.asarray(xs[t], np.float32)
        cols = []
        co = {}
        for rel in RELS_L[0]:
            s, r, d = rel
            p = lp[_k(*rel)]
            if s == t:
                w = np.asarray(p["W_src"], np.float32) @ np.asarray(p["a_src"], np.float32)
                co[("as",) + rel] = FEAT + len(cols)
                cols.append(x @ w)
            if d == t:
                w = np.asarray(p["W_dst"], np.float32) @ np.asarray(p["a_dst"], np.float32)
                co[("ad",) + rel] = FEAT + len(cols)
                cols.append(x @ w)
        tab = np.concatenate([x] + [c[:, None] for c in cols], axis=1)
        sent = np.zeros((1, tab.shape[1]), np.float32)
        for key, off in co.items():
            if key[0] == "as":
                sent[0, off] = SENT_AS
        tabs[t] = np.concatenate([tab, sent], axis=0).astype(np.float32)
        offs[t] = co
    return tabs, offs


def _ext2_layout(params):
    """Column offsets + W-col matrices for the layer-2 ext tables."""
    lp = params["layers"][1]
    offs = {}
    wcols = {}
    for t in TYPES:
        co = {}
        ws = []
        for rel in RELS_L[1]:
            s, r, d = rel
            p = lp[_k(*rel)]
            if s == t:
                co[("as",) + rel] = HID + len(ws)
                ws.append(np.asarray(p["W_src"], np.float32) @ np.asarray(p["a_src"], np.float32))
            if d == t:
                co[("ad",) + rel] = HID + len(ws)
                ws.append(np.asarray(p["W_dst"], np.float32) @ np.asarray(p["a_dst"], np.float32))
        offs[t] = co
        wcols[t] = (np.stack(ws, axis=1) if ws else np.zeros((HID, 0), np.float32))
    return offs, wcols


def build(meta):
    """Build the SPMD Bass program. meta carries all shapes/layout constants."""
    nc = bacc.Bacc(None, target_bir_lowering=False, debug=False, num_devices=N_CORES)
    dt = mybir.dt.float32
    g = meta["g"]
    W1 = meta["W1"]  # per type: ext1 width
    W2 = meta["W2"]  # per type: ext2 width
    cnt = meta["cnt"]  # per type: number of in-relations per layer
    lin1 = meta["lin1"]  # [385, 768] with bias row appended
    lin2 = meta["lin2"]  # [769, 2]

    ext1 = {t: nc.dram_tensor(f"ext1_{t}", [NODE_SIZES[t] + 1, W1[t]], dt,
                              kind="ExternalInput") for t in TYPES}
    sent2 = {t: nc.dram_tensor(f"sent2_{t}", [1, W2[t]], dt, kind="ExternalInput")
             for t in TYPES}
    wc2 = {t: nc.dram_tensor(f"wc2_{t}", [HID, max(1, W2[t] - HID)], dt,
                             kind="ExternalInput") for t in TYPES}
    # per-relation prescaled weight mats [din, HID] and bias rows [1, HID]
    wmat = {}
    brow = {}
    for li in (0, 1):
        din = FEAT if li == 0 else HID
        for rel in RELS_L[li]:
            nm = f"w{li}_" + _k(*rel)
            wmat[(li, rel)] = nc.dram_tensor(nm, [din, HID], dt, kind="ExternalInput")
            brow[(li, rel)] = nc.dram_tensor("b" + nm, [1, HID], dt, kind="ExternalInput")
    lin1_d = nc.dram_tensor("lin1", list(lin1.shape), dt, kind="ExternalInput")
    lin2_d = nc.dram_tensor("lin2", list(lin2.shape), dt, kind="ExternalInput")
    idx_d = {}
    for li in (0, 1):
        for rel in RELS_L[li]:
            ncols = meta["ncols"][(li, rel)]
            idx_d[(li, rel)] = nc.dram_tensor(f"idx{li}_" + _k(*rel), [P, ncols],
                                              mybir.dt.int32, kind="ExternalInput")
    # internal DRAM
    acc = {}
    for li in (0, 1):
        for rel in RELS_L[li]:
            sh = g[rel]["sh"]
            acc[(li, rel)] = nc.dram_tensor(f"acc{li}_" + _k(*rel), [sh + 1, HID], dt)
    shard2 = {t: nc.dram_tensor(f"shard2_{t}", [NODE_SIZES[t] // N_CORES + 2, W2[t]], dt)
              for t in TYPES}
    ext2 = {t: nc.dram_tensor(f"ext2_{t}", [NODE_SIZES[t] + 2 * N_CORES, W2[t]], dt,
                              addr_space="Shared") for t in TYPES}
    h2 = {t: nc.dram_tensor(f"h2_{t}", [NODE_SIZES[t] // N_CORES, HID], dt)
          for t in HEAD_TYPES}
    out_d = nc.dram_tensor("out", [NODE_SIZES["pending_transaction"] // N_CORES, OUT],
                           dt, kind="ExternalOutput")

    AL = mybir.AluOpType
    AF = mybir.ActivationFunctionType

    with tile.TileContext(nc) as tc:
        import contextlib
        with contextlib.ExitStack() as st:
            cpool = st.enter_context(tc.tile_pool(name="consts", bufs=1))
            gpool = st.enter_context(tc.tile_pool(name="gather", bufs=3))
            spool = st.enter_context(tc.tile_pool(name="work", bufs=3))
            opool = st.enter_context(tc.tile_pool(name="outs", bufs=3))
            ppool = st.enter_context(tc.tile_pool(name="psum", bufs=1, space="PSUM"))
            ipool = st.enter_context(tc.tile_pool(name="idx", bufs=2))

            ident = cpool.tile([P, P], dt)
            make_identity(nc, ident[:])
            ones1 = cpool.tile([1, P], dt)
            nc.vector.memset(ones1[:], 1.0)

            # preload per-relation W / bias rows
            wsb = {}
            bsb = {}
            for li in (0, 1):
                din = FEAT if li == 0 else HID
                for rel in RELS_L[li]:
                    w = cpool.tile([din, HID], dt, tag=f"w{li}_{rel[1]}_{rel[2]}")
                    nc.sync.dma_start(out=w[:], in_=wmat[(li, rel)][:])
                    b = cpool.tile([1, HID], dt, tag=f"b{li}_{rel[1]}_{rel[2]}")
                    nc.sync.dma_start(out=b[:], in_=brow[(li, rel)][:])
                    wsb[(li, rel)] = w
                    bsb[(li, rel)] = b

            def zero_dram(tgt, rows, width):
                z = cpool.tile([P, width], dt, tag="zz")
                nc.vector.memset(z[:], 0.0)
                nfull = rows // P
                if nfull:
                    nc.sync.dma_start(
                        out=tgt[: nfull * P, :].rearrange("(n p) w -> p n w", p=P),
                        in_=z[:].rearrange("p (o w) -> p o w", o=1)
                             .to_broadcast([P, nfull, width]))
                if rows % P:
                    nc.sync.dma_start(out=tgt[nfull * P: rows, :],
                                      in_=z[: rows % P, :])

            def edge_phase(li):
                din = FEAT if li == 0 else HID
                for rel in RELS_L[li]:
                    s, r, d = rel
                    G = g[rel]
                    sh = G["sh"]
                    zero_dram(acc[(li, rel)], sh + 1, HID)
                    Ws, Wd = (W1, W1) if li == 0 else (W2, W2)
                    exts = ext1 if li == 0 else ext2
                    as_off = meta["offs"][li][s][("as",) + rel]
                    ad_off = meta["offs"][li][d][("ad",) + rel]
                    ncols = meta["ncols"][(li, rel)]
                    idx_sb = ipool.tile([P, ncols], mybir.dt.int32, tag="idx")
                    nc.sync.dma_start(out=idx_sb[:], in_=idx_d[(li, rel)][:])
                    col = 0
                    for t in range(G["ntiles"]):
                        md = int(G["maxdeg"][t])
                        ws_t = Ws[s]
                        g_all = gpool.tile([P, md * ws_t], dt, tag="gall")
                        ad_g = gpool.tile([P, Wd[d]], dt, tag="adg")
                        if md > 1:
                            nc.gpsimd.indirect_dma_start(
                                out=ad_g[:], out_offset=None, in_=exts[d][:],
                                in_offset=bass.IndirectOffsetOnAxis(
                                    ap=idx_sb[:, col:col + 1], axis=0))
                        col += 1
                        for j in range(md):
                            nc.gpsimd.indirect_dma_start(
                                out=g_all[:, j * ws_t:(j + 1) * ws_t],
                                out_offset=None, in_=exts[s][:],
                                in_offset=bass.IndirectOffsetOnAxis(
                                    ap=idx_sb[:, col + j:col + j + 1], axis=0))
                        col += md
                        g3 = g_all[:].rearrange("p (j w) -> p w j", w=ws_t)
                        if md == 1:
                            tr_in = g_all[:, :din]
                        else:
                            ec = spool.tile([P, md], dt, tag="ec")
                            nc.vector.tensor_scalar(
                                out=ec[:],
                                in0=g3[:, as_off, :],
                                scalar1=ad_g[:, ad_off:ad_off + 1],
                                scalar2=None, op0=AL.add)
                            ec2 = spool.tile([P, md], dt, tag="ec2")
                            nc.vector.tensor_scalar_mul(ec2[:], ec[:], NEG_SLOPE)
                            nc.vector.tensor_tensor(out=ec[:], in0=ec[:],
                                                    in1=ec2[:], op=AL.max)
                            nc.scalar.activation(ec[:], ec[:], AF.Exp)
                            tmp = spool.tile([P, din * md], dt, tag="tmp")
                            nc.vector.tensor_tensor(
                                out=tmp[:], in0=g3[:, :din, :],
                                in1=ec[:].rearrange("p (o j) -> p o j", o=1)
                                       .to_broadcast([P, din, md]),
                                op=AL.mult)
                            num = spool.tile([P, din], dt, tag="num")
                            nc.vector.tensor_reduce(
                                out=num[:],
                                in_=tmp[:].rearrange("p (w j) -> p w j", j=md),
                                axis=mybir.AxisListType.X, op=AL.add)
                            den = spool.tile([P, 1], dt, tag="den")
                            nc.vector.tensor_reduce(out=den[:], in_=ec[:],
                                                    axis=mybir.AxisListType.X,
                                                    op=AL.add)
                            nc.vector.tensor_scalar_add(den[:], den[:], 1e-16)
                            rec = spool.tile([P, 1], dt, tag="rec")
                            nc.vector.reciprocal(rec[:], den[:])
                            numn = spool.tile([P, din], dt, tag="numn")
                            nc.vector.tensor_scalar(out=numn[:], in0=num[:],
                                                    scalar1=rec[:], scalar2=None,
                                                    op0=AL.mult)
                            tr_in = numn[:]
                        ntp = ppool.tile([din, P], dt, tag="ntp")
                        nc.tensor.transpose(ntp[:], tr_in, ident[:])
                        nts = spool.tile([din, P], dt, tag="nts")
                        nc.scalar.copy(nts[:], ntp[:])
                        ops = ppool.tile([P, HID], dt, tag="ops")
                        nc.tensor.matmul(ops[:], lhsT=nts[:], rhs=wsb[(li, rel)][:],
                                         start=True, stop=False)
                        nc.tensor.matmul(ops[:], lhsT=ones1[:],
                                         rhs=bsb[(li, rel)][:], start=False,
                                         stop=True)
                        osb = opool.tile([P, HID], dt, tag="osb")
                        nc.vector.tensor_copy(osb[:], ops[:])
                        nc.gpsimd.indirect_dma_start(
                            out=acc[(li, rel)][:], in_=osb[:],
                            out_offset=bass.IndirectOffsetOnAxis(
                                ap=idx_sb[:, col:col + 1], axis=0),
                            in_offset=None)
                        col += 1

            def merge_phase(li):
                # types to produce: layer0 -> all 6 (into shard2), layer1 -> head types
                ts = TYPES if li == 0 else HEAD_TYPES
                for t in ts:
                    rels = [rel for rel in RELS_L[li] if rel[2] == t]
                    sh = NODE_SIZES[t] // N_CORES
                    wcs = None
                    if li == 0 and W2[t] > HID:
                        wcs = cpool.tile([HID, W2[t] - HID], dt, tag="wc2")
                        nc.sync.dma_start(out=wcs[:], in_=wc2[t][:])
                    for t0 in range(0, sh, P):
                        rows = min(P, sh - t0)
                        h = spool.tile([P, HID], dt, tag="mh")
                        a0 = spool.tile([P, HID], dt, tag="ma")
                        nc.sync.dma_start(out=a0[:rows],
                                          in_=acc[(li, rels[0])][t0:t0 + rows, :])
                        prev = a0
                        for rr in rels[1:]:
                            ai = spool.tile([P, HID], dt, tag="mai")
                            nc.sync.dma_start(out=ai[:rows],
                                              in_=acc[(li, rr)][t0:t0 + rows, :])
                            nxt = spool.tile([P, HID], dt, tag="max")
                            nc.vector.tensor_tensor(out=nxt[:rows], in0=prev[:rows],
                                                    in1=ai[:rows], op=AL.add)
                            prev = nxt
                        nc.scalar.activation(h[:rows], prev[:rows], AF.Tanh)
                        if li == 0:
                            nc.sync.dma_start(out=shard2[t][t0:t0 + rows, :HID],
                                              in_=h[:rows])
                            if W2[t] > HID:
                                hp = ppool.tile([P, P], dt, tag="mhp")
                                nc.tensor.transpose(hp[:], h[:], ident[:])
                                hts = spool.tile([P, P], dt, tag="mhts")
                                nc.scalar.copy(hts[:], hp[:])
                                cp = ppool.tile([P, W2[t] - HID], dt, tag="mcp")
                                nc.tensor.matmul(cp[:], lhsT=hts[:], rhs=wcs[:],
                                                 start=True, stop=True)
                                cs = spool.tile([P, W2[t] - HID], dt, tag="mcs")
                                nc.vector.tensor_copy(cs[:], cp[:])
                                nc.sync.dma_start(
                                    out=shard2[t][t0:t0 + rows, HID:],
                                    in_=cs[:rows])
                        else:
                            nc.sync.dma_start(out=h2[t][t0:t0 + rows, :],
                                              in_=h[:rows])
                    if li == 0:
                        nc.sync.dma_start(out=shard2[t][sh:sh + 1, :],
                                          in_=sent2[t][:])

            edge_phase(0)
            merge_phase(0)
            for t in TYPES:
                nc.gpsimd.collective_compute(
                    "AllGather", AL.bypass,
                    ins=[shard2[t][:]],
                    outs=[ext2[t][:]],
                    replica_groups=[list(range(N_CORES))])
            edge_phase(1)
            merge_phase(1)

            # ----- head -----
            # load lin1 (385 x 768) as 4 lhsT chunks: rows 0:128,128:256,256:384,384:385
            l1c = []
            for kchunk in range(3):
                c0 = cpool.tile([P, 768], dt, tag=f"l1c{kchunk}")
                nc.sync.dma_start(out=c0[:], in_=lin1_d[kchunk * P:(kchunk + 1) * P, :])
                l1c.append(c0)
            b1c = cpool.tile([1, 768], dt, tag="l1b")
            nc.sync.dma_start(out=b1c[:], in_=lin1_d[384:385, :])
            l2c = []
            for m in range(6):
                c0 = cpool.tile([P, OUT], dt, tag=f"l2c{m}")
                nc.sync.dma_start(out=c0[:], in_=lin2_d[m * P:(m + 1) * P, :])
                l2c.append(c0)
            b2c = cpool.tile([1, OUT], dt, tag="l2b")
            nc.sync.dma_start(out=b2c[:], in_=lin2_d[768:769, :])

            shp = NODE_SIZES["pending_transaction"] // N_CORES
            for t0 in range(0, shp, P):
                rows = min(P, shp - t0)
                combT = []
                for t in HEAD_TYPES:
                    ht = spool.tile([P, HID], dt, tag="hh")
                    nc.sync.dma_start(out=ht[:rows], in_=h2[t][t0:t0 + rows, :])
                    hp = ppool.tile([P, P], dt, tag="hhp")
                    nc.tensor.transpose(hp[:], ht[:], ident[:])
                    hts = spool.tile([P, P], dt, tag="hhts")
                    nc.scalar.copy(hts[:], hp[:])
                    combT.append(hts)
                zts = []
                for m in range(6):
                    zp = ppool.tile([P, P], dt, tag="zp")
                    for kc in range(3):
                        nc.tensor.matmul(zp[:], lhsT=l1c[kc][:, m * P:(m + 1) * P],
                                         rhs=combT[kc][:], start=(kc == 0),
                                         stop=False)
                    nc.tensor.matmul(zp[:], lhsT=b1c[:, m * P:(m + 1) * P],
                                     rhs=ones1[:], start=False, stop=True)
                    zt = spool.tile([P, P], dt, tag="zt")
                    nc.scalar.activation(zt[:], zp[:], AF.Tanh)
                    zts.append(zt)
                op = ppool.tile([P, OUT], dt, tag="op")
                for m in range(6):
                    nc.tensor.matmul(op[:], lhsT=zts[m][:], rhs=l2c[m][:],
                                     start=(m == 0), stop=False)
                nc.tensor.matmul(op[:], lhsT=ones1[:], rhs=b2c[:], start=False,
                                 stop=True)
                ob = opool.tile([P, OUT], dt, tag="ob")
                nc.vector.tensor_copy(ob[:], op[:])
                nc.sync.dma_start(out=out_d[t0:t0 + rows, :], in_=ob[:rows])

    nc.compile()
    return nc


LAST = {}


def kernel(xs, edges, params):
    xs = {t: np.asarray(v, np.float32) for t, v in xs.items()}
    g = _prep_graph(edges)
    ext1_tabs, offs1 = _ext1_tables(xs, params)
    offs2, wcols2 = _ext2_layout(params)

    W1 = {t: ext1_tabs[t].shape[1] for t in TYPES}
    W2 = {t: HID + wcols2[t].shape[1] for t in TYPES}
    cnt = [{t: sum(1 for rel in RELS_L[li] if rel[2] == t) for t in TYPES}
           for li in (0, 1)]

    # prescaled W / bias rows (divide by per-type in-relation count of THAT layer's
    # FULL relation set -- reference divides by count over all 12 relations for
    # layer2 too... careful: reference layer2 uses all EDGE_TYPES for cnt)
    full_cnt = {t: sum(1 for s, r, d, _ in EDGE_TYPES if d == t) for t in TYPES}
    wmats = {}
    brows = {}
    for li in (0, 1):
        lp = params["layers"][li]
        for rel in RELS_L[li]:
            p = lp[_k(*rel)]
            cdiv = float(full_cnt[rel[2]])
            wmats[(li, rel)] = (np.asarray(p["W_src"], np.float32) / cdiv)
            brows[(li, rel)] = (np.asarray(p["bias"], np.float32) / cdiv)[None, :]

    lin1 = np.concatenate([np.asarray(params["lin1_w"], np.float32),
                           np.asarray(params["lin1_b"], np.float32)[None, :]], axis=0)
    lin2 = np.concatenate([np.asarray(params["lin2_w"], np.float32),
                           np.asarray(params["lin2_b"], np.float32)[None, :]], axis=0)

    sent2 = {}
    for t in TYPES:
        srow = np.zeros((1, W2[t]), np.float32)
        for key, off in offs2[t].items():
            if key[0] == "as":
                srow[0, off] = SENT_AS
        sent2[t] = srow

    idx0 = _idx_columns(g, 0)
    idx1 = _idx_columns(g, 1)
    ncols = {}
    for li, ii in ((0, idx0), (1, idx1)):
        for rel in RELS_L[li]:
            ncols[(li, rel)] = ii[0][rel].shape[1]

    meta = dict(g=g, W1=W1, W2=W2, cnt=cnt, lin1=lin1, lin2=lin2,
                offs={0: offs1, 1: offs2}, ncols=ncols)
    nc = build(meta)

    in_maps = []
    for c in range(N_CORES):
        m = {}
        for t in TYPES:
            m[f"ext1_{t}"] = ext1_tabs[t]
            m[f"sent2_{t}"] = sent2[t]
            m[f"wc2_{t}"] = (wcols2[t] if wcols2[t].shape[1] else
                             np.zeros((HID, 1), np.float32))
        for li in (0, 1):
            for rel in RELS_L[li]:
                m[f"w{li}_" + _k(*rel)] = wmats[(li, rel)]
                m["bw" + f"{li}_" + _k(*rel)] = brows[(li, rel)]
        m["lin1"] = lin1
        m["lin2"] = lin2
        for rel in RELS_L[0]:
            m["idx0_" + _k(*rel)] = idx0[c][rel]
        for rel in RELS_L[1]:
            m["idx1_" + _k(*rel)] = idx1[c][rel]
        in_maps.append(m)

    LAST.clear()
    LAST["nc"] = nc
    LAST["in_maps"] = in_maps
    res = run_bass_kernel_spmd(nc, in_maps, core_ids=list(range(N_CORES)))
    out = np.concatenate([res.results[c]["out"] for c in range(N_CORES)], axis=0)
    return out
